# revision 13
# baseline (speedup 1.0000x reference)
"""EnhancedGCNII on 8 Trainium2 NeuronCores — V3.

Strategy (row-sharded nodes, host-transposed fp8 adjacency):
  - Core c owns node rows Rc = [c*1024, (c+1)*1024).
  - Host ships AT_c = adj[Rc, :].T as fp8e4 (adj is exactly 0/1, fp8 exact)
    in partition-major layout [128, 64*1024]: partition p, free = kc*1024+r
    holds adj[r0+r, kc*128+p].  64KB contiguous per partition -> fat DMA
    descriptors, ~8MB/core streamed once.
  - Degrees: deg[r] = colsum of AT via an all-ones stationary fp8 DoubleRow
    matmul streamed over the same AT chunks, interleaved with the adj DMA.
    The output is duplicated across partitions, so rsqrt(deg+1) computed on
    the full [128,1024] tile directly yields the broadcast dinv tile.
  - Associativity: A_hat @ (H W') = (A_hat @ H) @ W', so per layer only ONE
    128-feature SpMM is needed: S^T = P'^T @ AT with P' = dinv*h gathered
    (fp8, split into two 512KB AllGathers).  Then AH = dinv*S + dinv^2*h,
    linear = AH@W'+b', gcnii = relu((AH + a/(1-a)*h0) @ (1-a)M + bg) with
    M = (1-b)I + b*Wg.
  - SpMM consumes gathered chunks half-by-half (local nodes 0-511 of every
    core first) so the next layer's matmuls start as soon as the first
    half-gather lands; epilogue halves are interleaved into the SpMM.
  - Everything stays in transposed [feat, node] layout; biases are
    per-partition scalars on the scalar engine.
  - Output: logits^T = fc_out_w^T @ h^T computed locally, host transposes.
"""

import sys
import types

sys.path.insert(0, "/opt/trn_rl_repo")

# ---------------------------------------------------------------------------
# Environment shims (axon container):
#  - antenv.axon_hooks is absent; register the NTFF profile hook ourselves so
#    trace=True yields exec_time_ns.
#  - no artifact bucket; skip uploads.
#  - walrus in this container allows only ONE semaphore wait on the CTRL
#    instruction Tile emits as the kernel-tail drain; split the waits across
#    sequential NOPs.
# ---------------------------------------------------------------------------
import antenv  # noqa: E402

if "antenv.axon_hooks" not in sys.modules:
    _mod = types.ModuleType("antenv.axon_hooks")
    _hook = [None]
    _mod.set_axon_ntff_profile_hook = lambda h: _hook.__setitem__(0, h)
    _mod.get_axon_ntff_profile_hook = lambda: _hook[0]
    sys.modules["antenv.axon_hooks"] = _mod
    antenv.axon_hooks = _mod
    try:
        from trn_agent_boot.trn_boot import _ntff_profile_via_ctypes

        _mod.set_axon_ntff_profile_hook(
            _ntff_profile_via_ctypes("/opt/axon/libaxon_pjrt.so")
        )
    except Exception as _e:
        print(f"ntff hook registration failed: {_e}", file=sys.stderr)

import numpy as np  # noqa: E402
import ml_dtypes  # noqa: E402
import concourse.bass as bass  # noqa: E402
import concourse.bacc as bacc  # noqa: E402
import concourse.mybir as mybir  # noqa: E402
import concourse.tile as tile  # noqa: E402
from concourse import bass_utils  # noqa: E402

bass_utils.upload_artifacts = lambda tmpdir: f"local://{tmpdir}"

_MAX_DRAIN_WAITS = 1


def _split_drain_and_barrier(self, tick_clock, wait_clock):
    nc = self.nc
    carrier = nc.sync.nop(hint="drain_wait_carrier", nofuse=True)
    wait_clock.add_sem_waits(
        carrier.ins, tile.ScopedClock({None: tick_clock.global_clock})
    )
    si = carrier.ins.sync_info
    if si is not None and len(si.on_wait) > _MAX_DRAIN_WAITS:
        waits = list(si.on_wait)
        carrier.ins.sync_info = mybir.SyncInfo(
            on_wait=waits[:_MAX_DRAIN_WAITS], on_update=list(si.on_update)
        )
        for i in range(_MAX_DRAIN_WAITS, len(waits), _MAX_DRAIN_WAITS):
            extra = nc.sync.nop(hint="drain_wait_split", nofuse=True)
            extra.ins.sync_info = mybir.SyncInfo(
                on_wait=waits[i : i + _MAX_DRAIN_WAITS], on_update=[]
            )
    nc.sync.drain()
    nc.all_engine_barrier()
    assert self.sems is not None
    popped = nc._tile_sem_poison_stack.pop()
    assert popped is self._sem_poison
    nc.clear_and_free_semaphores(list(self.sems.allocated().values()))
    nc.all_engine_barrier()


tile.TileContext._drain_and_barrier = _split_drain_and_barrier

# ---------------------------------------------------------------------------
# Problem constants (hardcoded per the harness contract)
# ---------------------------------------------------------------------------
import math  # noqa: E402

N, NFEAT, NHID, NCLASS, NLAYERS = 8192, 500, 128, 40, 4
ALPHA, GAMMA, LAMBDA = 0.1, 0.1, 0.5
NCORES = 8
NLOC = N // NCORES  # 1024 local nodes per core
K = N // 128  # 64 global node chunks
RB = NLOC // 128  # 8 local row blocks
NFP = 512  # padded feature dim
NHALF = N // 2  # gathered nodes per half-gather

DEBUG_DUMPS = False

F32 = mybir.dt.float32
BF16 = mybir.dt.bfloat16
FP8 = mybir.dt.float8e4

FP8_NP = ml_dtypes.float8_e4m3
FP8_ONE = np.array([1.0], FP8_NP).view(np.uint8)[0]

# SpMM consumption order of chunk-pairs: local-half-0 chunks of every core
# (global chunks 8c+0..3 -> pairs 4c,4c+1) first, then half-1 pairs.
KP_ORDER = [4 * c + j for c in range(NCORES) for j in (0, 1)] + [
    4 * c + j for c in range(NCORES) for j in (2, 3)
]


def build_program():
    nc = bacc.Bacc(num_devices=NCORES)

    at_d = nc.dram_tensor("at_c", [128, K * NLOC], FP8, kind="ExternalInput")
    xt_d = nc.dram_tensor("xT_c", [NFP, NLOC], BF16, kind="ExternalInput")
    fcw_d = nc.dram_tensor("fc_in_w_p", [NFP, NHID], F32, kind="ExternalInput")
    fcb_d = nc.dram_tensor("fc_in_b", [NHID], F32, kind="ExternalInput")
    c_d = nc.dram_tensor("c_vec", [NHID], F32, kind="ExternalInput")
    wg_d = nc.dram_tensor("w_gcnii", [NLAYERS, NHID, NHID], F32, kind="ExternalInput")
    bg_d = nc.dram_tensor("b_gcnii", [NLAYERS, NHID], F32, kind="ExternalInput")
    wl_d = nc.dram_tensor("w_lin", [NLAYERS, NHID, NHID], F32, kind="ExternalInput")
    bl_d = nc.dram_tensor("b_lin", [NLAYERS, NHID], F32, kind="ExternalInput")
    fow_d = nc.dram_tensor("fc_out_w", [NHID, NCLASS], F32, kind="ExternalInput")
    fob_d = nc.dram_tensor("fc_out_b", [NCLASS], F32, kind="ExternalInput")
    out_t = nc.dram_tensor("out_t", [NCLASS, NLOC], F32, kind="ExternalOutput")
    if DEBUG_DUMPS:
        dbg_dinv = nc.dram_tensor("dbg_dinv", [1, NLOC], F32, kind="ExternalOutput")
        dbg_h0 = nc.dram_tensor("dbg_h0", [128, NLOC], F32, kind="ExternalOutput")
        dbg_psb = nc.dram_tensor("dbg_psb", [128, K * 128], FP8, kind="ExternalOutput")
        dbg_st = nc.dram_tensor("dbg_st", [128, NLOC], F32, kind="ExternalOutput")
        dbg_h1 = nc.dram_tensor("dbg_h1", [128, NLOC], F32, kind="ExternalOutput")

    ident_d = nc.inline_tensor(np.eye(128, dtype=np.float32), name="ident128")

    betas = [math.log(LAMBDA / (i + 1) + 1.0) for i in range(NLAYERS)]

    with tile.TileContext(nc, num_cores=NCORES) as tc:
        with (
            tc.tile_pool(name="persist", bufs=1) as pp,
            tc.tile_pool(name="state", bufs=2) as stp,
            tc.tile_pool(name="psbp", bufs=2) as psbp,
            tc.tile_pool(name="dram", bufs=1, space="DRAM") as dram,
        ):
            # ---- persistent SBUF tiles (small loads on the ACT DMA queue) --
            at_all = pp.tile([128, K * NLOC], FP8)  # 64KB/partition
            ident = pp.tile([128, 128], F32)
            nc.scalar.dma_start(ident[:], ident_d[:])
            ident_bf = pp.tile([128, 128], BF16)
            nc.vector.tensor_copy(ident_bf[:], ident[:])
            ones_fp8 = pp.tile([128, 256], FP8)
            nc.vector.memset(ones_fp8[:], 1.0)

            fcw_sb = pp.tile([128, 4 * 128], F32)
            nc.scalar.dma_start(
                fcw_sb[:].rearrange("p (j f) -> p j f", j=4),
                fcw_d[:].rearrange("(j p) f -> p j f", p=128),
            )
            fcb_sb = pp.tile([128, 1], F32)
            nc.scalar.dma_start(fcb_sb[:], fcb_d[:].rearrange("(p o) -> p o", o=1))
            c_sb = pp.tile([128, 1], F32)
            nc.scalar.dma_start(c_sb[:], c_d[:].rearrange("(p o) -> p o", o=1))
            wg_sb = pp.tile([128, NLAYERS * 128], F32)
            wl_sb = pp.tile([128, NLAYERS * 128], F32)
            bg_sb = pp.tile([128, NLAYERS], F32)
            bl_sb = pp.tile([128, NLAYERS], F32)
            fow_sb = pp.tile([128, NCLASS], F32)
            fob_sb = pp.tile([NCLASS, 1], F32)

            ones_col = pp.tile([128, 1], F32)
            nc.vector.memset(ones_col[:], 1.0)
            c01 = pp.tile([128, 1], F32)
            nc.vector.tensor_scalar_mul(c01[:], c_sb[:], GAMMA)
            fcw_bf = pp.tile([128, 4 * 128], BF16)
            nc.vector.tensor_copy(fcw_bf[:], fcw_sb[:])
            fow_bf = pp.tile([128, NCLASS], BF16)
            wl_bf = pp.tile([128, NLAYERS * 128], BF16)
            m_bf = pp.tile([128, NLAYERS * 128], BF16)

            dinv_nch = pp.tile([128, RB], F32)
            b_d1 = pp.tile([128, NLOC], F32)  # dinv broadcast to all partitions
            b_d2 = pp.tile([128, NLOC], F32)
            h0T_01s = pp.tile([128, NLOC], F32)  # (ALPHA/(1-ALPHA)) * h0^T

            # AT chunk-pair views: free index = kc*1024 + r
            at_v = at_all[:].rearrange("p (kp o r) -> p kp o r", kp=K // 2, o=2)
            ones_dr = ones_fp8[:].rearrange("p (o m) -> p o m", o=2)

            # =============== phase 0 ===============
            hT = stp.tile([128, NLOC], BF16, tag="hT", name="hT_l0")
            ploc0 = pp.tile([128, RB * 128], FP8, name="ploc0")
            psb = psbp.tile([128, K * 128], FP8, tag="psb", name="psb0")
            with (
                tc.tile_pool(name="fcpool", bufs=1) as fcp,
                tc.tile_pool(name="ps_fc", bufs=2, space="PSUM") as psfc,
                tc.tile_pool(name="ps_deg", bufs=1, space="PSUM") as psdeg,
                tc.tile_pool(name="ps_p0", bufs=1, space="PSUM") as psp0,
            ):
                # x^T load (bf16, [feat 128p x 4j, 1024 nodes]) then AT in 4
                # partition-contiguous slabs, 16KB per partition each
                x_sb = fcp.tile([128, 4 * NLOC], BF16)
                nc.scalar.dma_start(
                    x_sb[:].rearrange("p (j n) -> p j n", j=4),
                    xt_d[:].rearrange("(j p) n -> p j n", p=128),
                )
                for g in range(4):
                    sl = slice(g * 16 * NLOC, (g + 1) * 16 * NLOC)
                    eng = nc.sync if g % 2 == 0 else nc.scalar
                    eng.dma_start(at_all[:, sl], at_d[:, sl])
                # remaining (late-needed) weights, after the big streams
                nc.scalar.dma_start(
                    wg_sb[:].rearrange("p (l f) -> p l f", l=NLAYERS),
                    wg_d[:].rearrange("l p f -> p l f"),
                )
                nc.scalar.dma_start(
                    wl_sb[:].rearrange("p (l f) -> p l f", l=NLAYERS),
                    wl_d[:].rearrange("l p f -> p l f"),
                )
                nc.scalar.dma_start(bg_sb[:], bg_d[:].rearrange("l p -> p l"))
                nc.scalar.dma_start(bl_sb[:], bl_d[:].rearrange("l p -> p l"))
                nc.scalar.dma_start(fow_sb[:], fow_d[:])
                nc.scalar.dma_start(
                    fob_sb[:], fob_d[:].rearrange("(p o) -> p o", o=1)
                )
                nc.vector.tensor_copy(fow_bf[:], fow_sb[:])
                nc.vector.tensor_copy(wl_bf[:], wl_sb[:])
                # M'_i = (1-ALPHA) * (beta_i*wg_i + (1-beta_i)*I)  (bf16 lhsT)
                for i in range(NLAYERS):
                    mtmp = stp.tile([128, 128], F32, tag="mtmp")
                    nc.vector.tensor_scalar_mul(
                        mtmp[:],
                        wg_sb[:, i * 128 : (i + 1) * 128],
                        (1.0 - ALPHA) * betas[i],
                    )
                    mtmp2 = stp.tile([128, 128], F32, tag="mtmp2")
                    nc.vector.tensor_scalar_mul(
                        mtmp2[:], ident[:], (1.0 - ALPHA) * (1.0 - betas[i])
                    )
                    nc.vector.tensor_add(
                        m_bf[:, i * 128 : (i + 1) * 128], mtmp[:], mtmp2[:]
                    )

                # fc_in: h0^T = relu(W^T x^T + b) -> gamma blend
                for nh in range(2):
                    ps_h = psfc.tile([128, 512], F32, tag="psfc")
                    for j in range(4):
                        nc.tensor.matmul(
                            ps_h[:],
                            fcw_bf[:, j * 128 : (j + 1) * 128],
                            x_sb[:, j * NLOC + nh * 512 : j * NLOC + (nh + 1) * 512],
                            start=(j == 0),
                            stop=(j == 3),
                        )
                    htmp = fcp.tile([128, 512], F32, tag="htmp", bufs=2)
                    nc.scalar.activation(
                        htmp[:],
                        ps_h[:],
                        mybir.ActivationFunctionType.Relu,
                        bias=fcb_sb[:, 0:1],
                    )
                    nc.scalar.activation(
                        hT[:, nh * 512 : (nh + 1) * 512],
                        htmp[:],
                        mybir.ActivationFunctionType.Identity,
                        bias=c01[:, 0:1],
                        scale=1.0 - GAMMA,
                    )
                nc.vector.tensor_scalar_mul(h0T_01s[:], hT[:], ALPHA / (1.0 - ALPHA))

                # degrees: deg[r] = sum_k AT[k, r] via all-ones stationary DR,
                # consuming the AT slabs as they land
                deg0 = psdeg.tile([128, 512], F32, name="deg0")
                deg1 = psdeg.tile([128, 512], F32, name="deg1")
                for kp in range(K // 2):
                    for rh in range(2):
                        nc.tensor.matmul(
                            (deg0 if rh == 0 else deg1)[:],
                            ones_dr,
                            at_v[:, kp, :, rh * 512 : (rh + 1) * 512],
                            start=(kp == 0),
                            stop=(kp == K // 2 - 1),
                            perf_mode=mybir.MatmulPerfMode.DoubleRow,
                        )
                # b_d1 = rsqrt(deg+1) on the partition-duplicated tile
                nc.scalar.activation(
                    b_d1[:, 0:512],
                    deg0[:],
                    mybir.ActivationFunctionType.Sqrt,
                    bias=ones_col[:, 0:1],
                )
                nc.scalar.activation(
                    b_d1[:, 512:1024],
                    deg1[:],
                    mybir.ActivationFunctionType.Sqrt,
                    bias=ones_col[:, 0:1],
                )
                nc.vector.reciprocal(b_d1[:], b_d1[:])

                # h0 transposes on the now-idle PE
                ps_tr = psp0.tile([128, NLOC], F32, name="ps_tr0")
                for nb in range(RB):
                    nc.tensor.matmul(
                        ps_tr[:, nb * 128 : (nb + 1) * 128],
                        hT[:, nb * 128 : (nb + 1) * 128],
                        ident_bf[:],
                        start=True,
                        stop=True,
                        skip_group_check=True,
                    )
                # dinv_nch via PE column extract of b_d1 (row 0 of each chunk)
                b_d1_bf = fcp.tile([128, NLOC], BF16, name="b_d1_bf")
                nc.vector.tensor_copy(b_d1_bf[:], b_d1[:])
                ps_nch = psp0.tile([128, RB], F32, name="ps_nch")
                for nb in range(RB):
                    nc.tensor.matmul(
                        ps_nch[:, nb : nb + 1],
                        b_d1_bf[:, nb * 128 : (nb + 1) * 128],
                        ident_bf[:, 0:1],
                        start=True,
                        stop=True,
                        skip_group_check=True,
                    )
                nc.vector.tensor_copy(dinv_nch[:], ps_nch[:])

                # ploc0 = dinv * h0 (node-major fp8), gathered per half
                for hf in range(2):
                    for nb in range(hf * 4, hf * 4 + 4):
                        nc.vector.tensor_scalar_mul(
                            ploc0[:, nb * 128 : (nb + 1) * 128],
                            ps_tr[:, nb * 128 : (nb + 1) * 128],
                            dinv_nch[:, nb : nb + 1],
                        )
                    cc_in = dram.tile([128, 4 * NHID], FP8, name=f"ccin0{hf}")
                    cc_out = dram.tile(
                        [128 * NCORES, 4 * NHID],
                        FP8,
                        addr_space="Shared",
                        name=f"ccout0{hf}",
                    )
                    nc.gpsimd.dma_start(
                        cc_in[:], ploc0[:, hf * 512 : (hf + 1) * 512]
                    )
                    nc.gpsimd.collective_compute(
                        "AllGather",
                        mybir.AluOpType.bypass,
                        replica_groups=[list(range(NCORES))],
                        ins=[cc_in[:].opt()],
                        outs=[cc_out[:].opt()],
                    )
                    nc.sync.dma_start(
                        psb[:].rearrange("p (cg hh b) -> p cg hh b", cg=NCORES, hh=2)[
                            :, :, hf, :
                        ],
                        cc_out[:].rearrange("(cg p) b -> p cg b", p=128),
                    )
                nc.vector.tensor_mul(b_d2[:], b_d1[:], b_d1[:])
                if DEBUG_DUMPS:
                    nc.scalar.dma_start(dbg_dinv[:], b_d1[0:1, :])
                    nc.scalar.dma_start(dbg_h0[:], hT[:])

            # =============== layers ===============
            with (
                tc.tile_pool(name="tmp4", bufs=4) as tp,
                tc.tile_pool(name="bfp", bufs=4) as bfp,
                tc.tile_pool(name="ps_st", bufs=1, space="PSUM") as pst,
                tc.tile_pool(name="ps_aux", bufs=2, space="PSUM") as psa,
                tc.tile_pool(name="ps_tr", bufs=2, space="PSUM") as pstr,
            ):
                psb_cur = psb
                for i in range(NLAYERS):
                    psb_v = psb_cur[:].rearrange(
                        "p (kp o f) -> p kp o f", kp=K // 2, o=2
                    )
                    if DEBUG_DUMPS and i == 0:
                        nc.sync.dma_start(dbg_psb[:], psb_cur[:])
                    # t2 = dinv^2 * h — no SpMM dependency, DVE does it early
                    t2 = tp.tile([128, NLOC], F32, tag="t2", bufs=2, name=f"t2_{i}")
                    nc.vector.tensor_mul(t2[:], hT[:], b_d2[:])

                    hT_new = stp.tile([128, NLOC], BF16, tag="hT", name=f"hT_l{i + 1}")
                    if i < NLAYERS - 1:
                        ploc_next = tp.tile(
                            [128, RB * 128], FP8, tag="ploc", bufs=2,
                            name=f"ploc{i + 1}",
                        )
                        psb_next = psbp.tile(
                            [128, K * 128], FP8, tag="psb", name=f"psb{i + 1}"
                        )

                    st_tiles = [
                        pst.tile([128, 512], F32, tag=f"st{rh}", name=f"st{rh}_{i}")
                        for rh in range(2)
                    ]
                    lin_tiles = [None, None]
                    gc_tiles = [None, None]

                    def spmm(rh, j0, j1, psb_v=psb_v, st_tiles=st_tiles):
                        for j in range(j0, j1):
                            kp = KP_ORDER[j]
                            nc.tensor.matmul(
                                st_tiles[rh][:],
                                psb_v[:, kp],
                                at_v[:, kp, :, rh * 512 : (rh + 1) * 512],
                                start=(j == 0),
                                stop=(j == K // 2 - 1),
                                perf_mode=mybir.MatmulPerfMode.DoubleRow,
                            )

                    def epi_front(rh, i=i, st_tiles=st_tiles, t2=t2):
                        st = st_tiles[rh]
                        sl = slice(rh * 512, (rh + 1) * 512)
                        t1 = tp.tile([128, 512], F32, tag="t1", name=f"t1_{i}_{rh}")
                        nc.vector.tensor_mul(t1[:], st[:], b_d1[:, sl])
                        ah_bf = bfp.tile(
                            [128, 512], BF16, tag="ah", name=f"ah_{i}_{rh}"
                        )
                        nc.vector.tensor_add(ah_bf[:], t1[:], t2[:, sl])
                        u_bf = bfp.tile([128, 512], BF16, tag="u", name=f"u_{i}_{rh}")
                        nc.vector.tensor_add(u_bf[:], ah_bf[:], h0T_01s[:, sl])
                        return ah_bf, u_bf

                    def epi_mm(rh, ah_bf, u_bf, i=i):
                        ps_lin = psa.tile(
                            [128, 512], F32, tag="aux", name=f"pl_{i}_{rh}"
                        )
                        nc.tensor.matmul(
                            ps_lin[:],
                            wl_bf[:, i * 128 : (i + 1) * 128],
                            ah_bf[:],
                            start=True,
                            stop=True,
                        )
                        ps_gc = psa.tile(
                            [128, 512], F32, tag="aux", name=f"pg_{i}_{rh}"
                        )
                        nc.tensor.matmul(
                            ps_gc[:],
                            m_bf[:, i * 128 : (i + 1) * 128],
                            u_bf[:],
                            start=True,
                            stop=True,
                        )
                        return ps_lin, ps_gc

                    def epi_act(rh, ps_lin, ps_gc, i=i, hT_new=hT_new):
                        sl = slice(rh * 512, (rh + 1) * 512)
                        lin_sb = bfp.tile(
                            [128, 512], BF16, tag="lin", name=f"ls_{i}_{rh}"
                        )
                        nc.scalar.activation(
                            lin_sb[:],
                            ps_lin[:],
                            mybir.ActivationFunctionType.Identity,
                            bias=bl_sb[:, i : i + 1],
                        )
                        gc_sb = bfp.tile(
                            [128, 512], BF16, tag="gc", name=f"gs_{i}_{rh}"
                        )
                        nc.scalar.activation(
                            gc_sb[:],
                            ps_gc[:],
                            mybir.ActivationFunctionType.Relu,
                            bias=bg_sb[:, i : i + 1],
                        )
                        nc.vector.tensor_add(hT_new[:, sl], lin_sb[:], gc_sb[:])

                    def epi_tr(rh, i=i, hT_new=hT_new):
                        ps_tr = pstr.tile(
                            [128, 512], F32, tag="tr", name=f"tr_{i}_{rh}"
                        )
                        for nb in range(4):
                            nc.tensor.matmul(
                                ps_tr[:, nb * 128 : (nb + 1) * 128],
                                hT_new[
                                    :, rh * 512 + nb * 128 : rh * 512 + (nb + 1) * 128
                                ],
                                ident_bf[:],
                                start=True,
                                stop=True,
                                skip_group_check=True,
                            )
                        return ps_tr

                    def epi_gather(rh, ps_tr, i=i):
                        for nb in range(4):
                            gnb = rh * 4 + nb
                            nc.vector.tensor_scalar_mul(
                                ploc_next[:, gnb * 128 : (gnb + 1) * 128],
                                ps_tr[:, nb * 128 : (nb + 1) * 128],
                                dinv_nch[:, gnb : gnb + 1],
                            )
                        cc_in = dram.tile(
                            [128, 4 * NHID], FP8, name=f"ccin{i + 1}{rh}"
                        )
                        cc_out = dram.tile(
                            [128 * NCORES, 4 * NHID],
                            FP8,
                            addr_space="Shared",
                            name=f"ccout{i + 1}{rh}",
                        )
                        nc.gpsimd.dma_start(
                            cc_in[:], ploc_next[:, rh * 512 : (rh + 1) * 512]
                        )
                        nc.gpsimd.collective_compute(
                            "AllGather",
                            mybir.AluOpType.bypass,
                            replica_groups=[list(range(NCORES))],
                            ins=[cc_in[:].opt()],
                            outs=[cc_out[:].opt()],
                        )
                        nc.sync.dma_start(
                            psb_next[:]
                            .rearrange("p (cg hh b) -> p cg hh b", cg=NCORES, hh=2)[
                                :, :, rh, :
                            ],
                            cc_out[:].rearrange("(cg p) b -> p cg b", p=128),
                        )

                    last = i == NLAYERS - 1
                    spmm(0, 0, K // 2)
                    ah0, u0 = epi_front(0)
                    spmm(1, 0, 8)
                    pl0, pg0 = epi_mm(0, ah0, u0)
                    epi_act(0, pl0, pg0)
                    spmm(1, 8, 16)
                    if not last:
                        tr0 = epi_tr(0)
                    spmm(1, 16, K // 2)
                    if not last:
                        epi_gather(0, tr0)
                    ah1, u1 = epi_front(1)
                    pl1, pg1 = epi_mm(1, ah1, u1)
                    epi_act(1, pl1, pg1)
                    if not last:
                        tr1 = epi_tr(1)
                        epi_gather(1, tr1)
                    if DEBUG_DUMPS and i == 0:
                        dbg_s = tp.tile([128, NLOC], F32, tag="dbgs", name="dbgs")
                        nc.vector.tensor_copy(dbg_s[:, 0:512], st_tiles[0][:])
                        nc.vector.tensor_copy(dbg_s[:, 512:1024], st_tiles[1][:])
                        nc.sync.dma_start(dbg_st[:], dbg_s[:])
                        nc.sync.dma_start(dbg_h1[:], hT_new[:])
                    hT = hT_new
                    if i < NLAYERS - 1:
                        psb_cur = psb_next

                # ---- output head ----
                ps_o = psa.tile([NCLASS, NLOC], F32, tag="auxo", name="pso", bufs=1)
                for nh in range(2):
                    nc.tensor.matmul(
                        ps_o[:, nh * 512 : (nh + 1) * 512],
                        fow_bf[:, 0:NCLASS],
                        hT[:, nh * 512 : (nh + 1) * 512],
                        start=True,
                        stop=True,
                    )
                out_sb = tp.tile([NCLASS, NLOC], F32, tag="outsb", name="out_sb")
                nc.scalar.activation(
                    out_sb[:],
                    ps_o[:],
                    mybir.ActivationFunctionType.Identity,
                    bias=fob_sb[:, 0:1],
                )
                nc.sync.dma_start(out_t[:], out_sb[:])

    nc.compile()
    return nc


_program_cache = {}


def _get_program():
    if "nc" not in _program_cache:
        _program_cache["nc"] = build_program()
    return _program_cache["nc"]


def kernel(
    x,
    adj,
    fc_in_w,
    fc_in_b,
    c,
    w_gcnii,
    b_gcnii,
    w_lin,
    b_lin,
    fc_out_w,
    fc_out_b,
    _trace=False,
):
    x = np.asarray(x, dtype=np.float32)
    adj = np.asarray(adj, dtype=np.float32)
    x_pad = np.zeros((N, NFP), np.float32)
    x_pad[:, :NFEAT] = x
    xt = np.ascontiguousarray(x_pad.T).astype(ml_dtypes.bfloat16)  # [512, N]
    fcw_pad = np.zeros((NFP, NHID), np.float32)
    fcw_pad[:NFEAT, :] = np.asarray(fc_in_w, np.float32)
    # adj is exactly 0/1; re-encode losslessly as fp8e4 (1.0 = 0x38)
    adj8 = (adj.astype(np.uint8) * FP8_ONE).view(FP8_NP)

    shared = {
        "fc_in_w_p": fcw_pad,
        "fc_in_b": np.asarray(fc_in_b, np.float32),
        "c_vec": np.asarray(c, np.float32),
        "w_gcnii": np.ascontiguousarray(w_gcnii, np.float32),
        "b_gcnii": np.ascontiguousarray(b_gcnii, np.float32),
        "w_lin": np.ascontiguousarray(w_lin, np.float32),
        "b_lin": np.ascontiguousarray(b_lin, np.float32),
        "fc_out_w": np.ascontiguousarray(fc_out_w, np.float32),
        "fc_out_b": np.asarray(fc_out_b, np.float32),
    }
    in_maps = []
    for cix in range(NCORES):
        r0, r1 = cix * NLOC, (cix + 1) * NLOC
        m = dict(shared)
        # partition-major AT: [kc, p, r] -> [p, kc, r] flattened to [128, 64K]
        slab = adj8[r0:r1, :].T.reshape(K, 128, NLOC)
        m["at_c"] = np.ascontiguousarray(slab.transpose(1, 0, 2)).reshape(
            128, K * NLOC
        )
        m["xT_c"] = np.ascontiguousarray(xt[:, r0:r1])  # [512, NLOC] bf16
        in_maps.append(m)

    nc = _get_program()
    res = bass_utils.run_bass_kernel_spmd(
        nc, in_maps=in_maps, core_ids=list(range(NCORES)), trace=_trace
    )
    out = np.empty((N, NCLASS), np.float32)
    for cix in range(NCORES):
        out[cix * NLOC : (cix + 1) * NLOC, :] = res.results[cix]["out_t"].T
    kernel.last_exec_time_ns = res.exec_time_ns
    kernel.last_results = res
    return out


kernel.last_exec_time_ns = None
kernel.last_results = None
